# revision 1
# baseline (speedup 1.0000x reference)
"""BigBird block-sparse self-attention on 8 Trainium2 NeuronCores.

Reference semantics (B=4, l=4096, d=768, BLOCK=128):
  q,k,v = split(qkv); scores = q @ k^T / sqrt(d)
  mask: row i attends to j iff j<=i AND (j < 256 [global cols]
        OR j >= i-384 [sliding window] OR (i,j) in random 2x2 cells
        derived from pos0/pos1 pairs with pos0>pos1)
  out = softmax(scores + mask) @ v   (masked entries get <=-1e4, whose
        exp underflows to exactly 0 in fp32)

Strategy: 128-row blocks paired into 256-row "pair" units (N=256 matmul
free dim). 16 pairs x 4 batches = 64 units; 8 cores x 8 units each
(core c: batch c//2, pairs 8*(c%2)..8*(c%2)+7). Per unit the column
blocks are: 2 global blocks, 5 window blocks (2m-3..2m+1), and NE
gathered "extra" blocks holding the random cells not covered by
global/window. Scores are computed transposed (S^T[jj,ii]) so softmax
normalization comes out of the AV matmul via a ones-column appended to
V; masks are 0/1 multiplicative on P=exp(S^T) (underflow-equivalent to
the reference's additive -1e4). Matmuls run in bf16 with fp32 PSUM
accumulation; output is fp32.
"""

import os
import sys

for _p in ("/opt/trn_rl_repo", "/root/.axon_site/_ro/trn_rl_repo"):
    if _p not in sys.path and os.path.isdir(_p):
        sys.path.append(_p)

import ml_dtypes
import numpy as np

import concourse.bass as bass
import concourse.mybir as mybir
from bass_rust import InstNoOp
from concourse.bass_utils import run_bass_kernel_spmd
from concourse.tile import TileContext

BLOCK = 128
B, L, D = 4, 4096, 768
NB = L // BLOCK          # 32 column blocks
WINDOW = 3 * BLOCK       # 384
GLOBAL = 2 * BLOCK       # 256
NCHUNK = D // 128        # 6 contraction chunks
N_CORES = 8
UNITS = 8                # pairs per core
PAIR_ROWS = 2 * BLOCK    # 256
NWIN = 2 * UNITS + 3     # 19 window blocks cached per core

F32 = mybir.dt.float32
BF16 = mybir.dt.bfloat16
BF16_NP = ml_dtypes.bfloat16

_last_results = None     # test harness reads exec_time_ns from here


# ---------------------------------------------------------------------------
# walrus in this toolchain rejects >1 sync-wait command per instruction;
# split excess waits onto same-engine NoOps inserted just before.
def _split_excess_waits(nc, max_w=1):
    ctr = 0
    for blk in nc.m.functions[0].blocks:
        out = []
        changed = False
        for inst in blk.instructions:
            si = inst.sync_info
            waits = list(si.on_wait) if (si is not None and si.on_wait) else []
            if len(waits) > max_w:
                changed = True
                excess, keep = waits[:-max_w], waits[-max_w:]
                for i in range(0, len(excess), max_w):
                    ctr += 1
                    nop = InstNoOp(name=f"wsplit_{ctr}", ins=[], outs=[])
                    nop.engine = inst.engine
                    nop.sync_info = mybir.SyncInfo(
                        on_wait=excess[i : i + max_w], on_update=[]
                    )
                    out.append(nop)
                inst.sync_info = mybir.SyncInfo(
                    on_wait=keep, on_update=list(si.on_update or [])
                )
            out.append(inst)
        if changed:
            blk.instructions = out


# ---------------------------------------------------------------------------
# host-side data prep

def _transpose_block(x):
    """[rows, 768] -> [128, NCHUNK*rows] with [p, c*rows+r] = x[r, c*128+p]."""
    rows = x.shape[0]
    return np.ascontiguousarray(
        x.reshape(rows, NCHUNK, 128).transpose(2, 1, 0).reshape(128, NCHUNK * rows)
    )


def _static_masks():
    ii = np.arange(PAIR_ROWS)[None, :]
    jj = np.arange(128)[:, None]
    m1 = (jj <= ii)            # block 2m   : (TRI_L | FULL)
    m2 = (jj + 128 <= ii)      # block 2m+1 : (ZERO  | TRI_L)
    m3 = (jj >= ii - 128)      # block 2m-2 : (FULL  | TRI_U)
    m4 = (jj >= ii)            # block 2m-3 : (TRI_U | ZERO)
    return [m.astype(BF16_NP) for m in (m1, m2, m3, m4)]


def _extra_cells(pos0, pos1):
    """Random-attention cells not covered by global/window, deduped.
    Returns (I, J) row/col arrays (batch-independent)."""
    pos0 = np.asarray(pos0).astype(np.int64).ravel()
    pos1 = np.asarray(pos1).astype(np.int64).ravel()
    valid = pos0 > pos1
    p0, p1 = pos0[valid], pos1[valid]
    I = np.concatenate([p0, p0, p0 + 1, p0 + 1])
    J = np.concatenate([p1, p1 + 1, p1, p1 + 1])
    ok = (J <= I) & (I < L) & (J < L)
    covered = (J < GLOBAL) | (J >= I - WINDOW)
    keep = ok & ~covered
    I, J = I[keep], J[keep]
    lin = np.unique(I * L + J)
    return lin // L, lin % L


def _prepare(qkv, pos0, pos1):
    qkv = np.asarray(qkv, dtype=np.float32)
    q = qkv[:, :, 0:D]
    k = qkv[:, :, D : 2 * D]
    v = qkv[:, :, 2 * D : 3 * D]
    scale = 1.0 / float(np.sqrt(D))

    m1, m2, m3, m4 = _static_masks()
    m14 = np.stack([m1, m2, m3, m4])                       # [4,128,256]

    I, J = _extra_cells(pos0, pos1)
    pair_of = I // PAIR_ROWS
    # per-pair unique columns
    pair_cols = {}
    for m in range(L // PAIR_ROWS):
        sel = pair_of == m
        pair_cols[m] = np.unique(J[sel])
    e_max = max((len(c) for c in pair_cols.values()), default=0)
    NE = max(1, -(-e_max // 128))                          # extra slots per unit

    # per-pair data-independent extra masks [NE,128,256]
    pair_mx = {}
    for m, cols in pair_cols.items():
        mx = np.zeros((NE, 128, PAIR_ROWS), dtype=BF16_NP)
        if len(cols):
            sel = pair_of == m
            e = np.searchsorted(cols, J[sel])
            mx[e // 128, e % 128, I[sel] - m * PAIR_ROWS] = 1.0
        pair_mx[m] = mx

    in_maps = []
    for c in range(N_CORES):
        b, h = c // 2, c % 2
        kb, vb, qb = k[b], v[b], q[b]

        ktw = np.zeros((NWIN, 128, NCHUNK * 128), dtype=BF16_NP)
        vw = np.zeros((NWIN, 128, D + 1), dtype=BF16_NP)
        for j in range(NWIN):
            blk = 16 * h - 3 + j
            if 2 <= blk < NB:                 # blocks 0,1 are served by the
                r0 = blk * BLOCK              # global slots; <0 don't exist
                ktw[j] = _transpose_block(kb[r0 : r0 + BLOCK])
                vw[j, :, :D] = vb[r0 : r0 + BLOCK]
                vw[j, :, D] = 1.0

        ktg = np.concatenate(
            [_transpose_block(kb[0:BLOCK]), _transpose_block(kb[BLOCK:GLOBAL])], axis=1
        ).astype(BF16_NP)                                  # [128, 2*768]
        vg = np.zeros((128, 2 * (D + 1)), dtype=BF16_NP)
        vg[:, 0:D] = vb[0:BLOCK]
        vg[:, D] = 1.0
        vg[:, D + 1 : 2 * D + 1] = vb[BLOCK:GLOBAL]
        vg[:, 2 * D + 1] = 1.0

        qt = np.zeros((UNITS, 128, NCHUNK * PAIR_ROWS), dtype=BF16_NP)
        mg = np.ones((UNITS, 128, 2 * PAIR_ROWS), dtype=BF16_NP)
        kte = np.zeros((UNITS * NE, 128, NCHUNK * 128), dtype=BF16_NP)
        ve = np.zeros((UNITS * NE, 128, D + 1), dtype=BF16_NP)
        mx = np.zeros((UNITS * NE, 128, PAIR_ROWS), dtype=BF16_NP)
        for u in range(UNITS):
            m = 8 * h + u
            r0 = m * PAIR_ROWS
            qt[u] = _transpose_block(qb[r0 : r0 + PAIR_ROWS] * scale)
            if m == 0:  # pair (0,1): global cols need the causal triangle
                mg[u, :, 0:PAIR_ROWS] = m1
                mg[u, :, PAIR_ROWS:] = m2
            cols = pair_cols[m]
            for x in range(NE):
                cc = cols[x * 128 : (x + 1) * 128]
                if len(cc):
                    kx = np.zeros((128, D), dtype=np.float32)
                    vx = np.zeros((128, D + 1), dtype=np.float32)
                    kx[: len(cc)] = kb[cc]
                    vx[: len(cc), :D] = vb[cc]
                    vx[: len(cc), D] = 1.0
                    kte[u * NE + x] = _transpose_block(kx)
                    ve[u * NE + x] = vx
            mx[u * NE : (u + 1) * NE] = pair_mx[m]

        in_maps.append(
            {
                "qt": qt, "ktw": ktw, "vw": vw, "ktg": ktg, "vg": vg,
                "kte": kte, "ve": ve, "mx": mx, "mg": mg,
                "m14": m14.astype(BF16_NP),
            }
        )
    return in_maps, NE


# ---------------------------------------------------------------------------
# device program (identical across cores; all variation is in the data)

def _build_program(NE):
    nc = bass.Bass()
    d_qt = nc.dram_tensor("qt", [UNITS, 128, NCHUNK * PAIR_ROWS], BF16, kind="ExternalInput")
    d_ktw = nc.dram_tensor("ktw", [NWIN, 128, NCHUNK * 128], BF16, kind="ExternalInput")
    d_vw = nc.dram_tensor("vw", [NWIN, 128, D + 1], BF16, kind="ExternalInput")
    d_ktg = nc.dram_tensor("ktg", [128, 2 * NCHUNK * 128], BF16, kind="ExternalInput")
    d_vg = nc.dram_tensor("vg", [128, 2 * (D + 1)], BF16, kind="ExternalInput")
    d_kte = nc.dram_tensor("kte", [UNITS * NE, 128, NCHUNK * 128], BF16, kind="ExternalInput")
    d_ve = nc.dram_tensor("ve", [UNITS * NE, 128, D + 1], BF16, kind="ExternalInput")
    d_mx = nc.dram_tensor("mx", [UNITS * NE, 128, PAIR_ROWS], BF16, kind="ExternalInput")
    d_mg = nc.dram_tensor("mg", [UNITS, 128, 2 * PAIR_ROWS], BF16, kind="ExternalInput")
    d_m14 = nc.dram_tensor("m14", [4, 128, PAIR_ROWS], BF16, kind="ExternalInput")
    d_out = nc.dram_tensor("out", [UNITS, PAIR_ROWS, D], F32, kind="ExternalOutput")

    EXP = mybir.ActivationFunctionType.Exp

    with TileContext(nc) as tc:
        with (
            tc.tile_pool(name="const", bufs=1) as const_pool,
            tc.tile_pool(name="ktw", bufs=8) as ktw_pool,
            tc.tile_pool(name="vw", bufs=8) as vw_pool,
            tc.tile_pool(name="qt", bufs=2) as qt_pool,
            tc.tile_pool(name="ext", bufs=2 * NE + 2) as ext_pool,
            tc.tile_pool(name="mgp", bufs=2) as mg_pool,
            tc.tile_pool(name="pt", bufs=12) as pt_pool,
            tc.tile_pool(name="eps", bufs=4) as eps_pool,
            tc.tile_pool(name="ob", bufs=3) as ob_pool,
            tc.tile_pool(name="st", bufs=3, space="PSUM") as st_pool,
            tc.tile_pool(name="av", bufs=2, space="PSUM") as av_pool,
        ):
            m14_t = const_pool.tile([128, 4 * PAIR_ROWS], BF16)
            for i in range(4):
                nc.sync.dma_start(
                    out=m14_t[:, i * PAIR_ROWS : (i + 1) * PAIR_ROWS], in_=d_m14[i]
                )
            ktg_t = const_pool.tile([128, 2 * NCHUNK * 128], BF16)
            nc.sync.dma_start(out=ktg_t[:], in_=d_ktg[:])
            vg_t = const_pool.tile([128, 2 * (D + 1)], BF16)
            nc.sync.dma_start(out=vg_t[:], in_=d_vg[:])

            ktw_tiles, vw_tiles = {}, {}

            def mask_ap(kind, mg_t, mx_ts, idx):
                if kind == "m":
                    return m14_t[:, idx * PAIR_ROWS : (idx + 1) * PAIR_ROWS]
                if kind == "g":
                    return mg_t[:, idx * PAIR_ROWS : (idx + 1) * PAIR_ROWS]
                if kind == "x":
                    return mx_ts[idx][:]
                return None

            for u in range(UNITS):
                qt_t = qt_pool.tile([128, NCHUNK * PAIR_ROWS], BF16, name=f"qt{u}")
                nc.sync.dma_start(out=qt_t[:], in_=d_qt[u])
                mg_t = mg_pool.tile([128, 2 * PAIR_ROWS], BF16, name=f"mg{u}")
                nc.sync.dma_start(out=mg_t[:], in_=d_mg[u])
                for j in range(2 * u, 2 * u + 5):
                    if j not in ktw_tiles:
                        kt = ktw_pool.tile([128, NCHUNK * 128], BF16, tag="ktw", name=f"ktw{j}")
                        nc.sync.dma_start(out=kt[:], in_=d_ktw[j])
                        ktw_tiles[j] = kt
                        vt = vw_pool.tile([128, D + 1], BF16, tag="vw", name=f"vw{j}")
                        nc.sync.dma_start(out=vt[:], in_=d_vw[j])
                        vw_tiles[j] = vt
                mx_ts, kte_ts, ve_ts = [], [], []
                for x in range(NE):
                    mt = ext_pool.tile([128, PAIR_ROWS], BF16, tag="mx", name=f"mx{u}_{x}")
                    nc.sync.dma_start(out=mt[:], in_=d_mx[u * NE + x])
                    mx_ts.append(mt)
                    ket = ext_pool.tile([128, NCHUNK * 128], BF16, tag="kte", name=f"kte{u}_{x}")
                    nc.sync.dma_start(out=ket[:], in_=d_kte[u * NE + x])
                    kte_ts.append(ket)
                    vet = ext_pool.tile([128, D + 1], BF16, tag="ve", name=f"ve{u}_{x}")
                    nc.sync.dma_start(out=vet[:], in_=d_ve[u * NE + x])
                    ve_ts.append(vet)

                # slot list: (kT access, V access, mask kind/idx)
                slots = [
                    (ktg_t[:, 0 : NCHUNK * 128], vg_t[:, 0 : D + 1], ("g", 0)),
                    (ktg_t[:, NCHUNK * 128 :], vg_t[:, D + 1 :], ("g", 1)),
                    (ktw_tiles[2 * u][:], vw_tiles[2 * u][:], ("m", 3)),       # M4
                    (ktw_tiles[2 * u + 1][:], vw_tiles[2 * u + 1][:], ("m", 2)),  # M3
                    (ktw_tiles[2 * u + 2][:], vw_tiles[2 * u + 2][:], None),   # full
                    (ktw_tiles[2 * u + 3][:], vw_tiles[2 * u + 3][:], ("m", 0)),  # M1
                    (ktw_tiles[2 * u + 4][:], vw_tiles[2 * u + 4][:], ("m", 1)),  # M2
                ] + [(kte_ts[x][:], ve_ts[x][:], ("x", x)) for x in range(NE)]

                av = [
                    av_pool.tile([128, D + 1], F32, tag="av", name=f"av{u}_{hh}")
                    for hh in range(2)
                ]
                n_s = len(slots)
                for si, (kt_ap, v_ap, mk) in enumerate(slots):
                    st = st_pool.tile([128, PAIR_ROWS], F32, tag="st", name=f"st{u}_{si}")
                    for cc in range(NCHUNK):
                        nc.tensor.matmul(
                            st[:],
                            kt_ap[:, cc * 128 : (cc + 1) * 128],
                            qt_t[:, cc * PAIR_ROWS : (cc + 1) * PAIR_ROWS],
                            start=(cc == 0),
                            stop=(cc == NCHUNK - 1),
                        )
                    pt = pt_pool.tile([128, PAIR_ROWS], BF16, tag="pt", name=f"pt{u}_{si}")
                    nc.scalar.activation(pt[:], st[:], EXP)
                    m_ap = mask_ap(mk[0], mg_t, mx_ts, mk[1]) if mk else None
                    if m_ap is not None:
                        nc.vector.tensor_mul(pt[:], pt[:], m_ap)
                    for hh in range(2):
                        lhs = pt[:, hh * 128 : (hh + 1) * 128]
                        nc.tensor.matmul(
                            av[hh][:, 0:512], lhs, v_ap[:, 0:512],
                            start=(si == 0), stop=(si == n_s - 1),
                        )
                        nc.tensor.matmul(
                            av[hh][:, 512 : D + 1], lhs, v_ap[:, 512 : D + 1],
                            start=(si == 0), stop=(si == n_s - 1),
                        )
                for hh in range(2):
                    rc = eps_pool.tile([128, 1], F32, tag="rc", name=f"rc{u}_{hh}")
                    nc.vector.reciprocal(rc[:], av[hh][:, D : D + 1])
                    ob = ob_pool.tile([128, D], F32, tag="ob", name=f"ob{u}_{hh}")
                    nc.scalar.mul(ob[:], av[hh][:, 0:D], rc[:])
                    nc.sync.dma_start(
                        out=d_out[u, hh * 128 : (hh + 1) * 128, :], in_=ob[:]
                    )

    _split_excess_waits(nc, max_w=1)
    return nc


_program_cache = {}


def kernel(qkv, pos0, pos1):
    global _last_results
    in_maps, NE = _prepare(qkv, pos0, pos1)
    if NE not in _program_cache:
        _program_cache[NE] = _build_program(NE)
    nc = _program_cache[NE]
    res = run_bass_kernel_spmd(
        nc, in_maps, core_ids=list(range(N_CORES)),
        trace=bool(os.environ.get("BASS_TRACE")),
    )
    _last_results = res
    out = np.empty((B, L, D), dtype=np.float32)
    for c in range(N_CORES):
        b, h = c // 2, c % 2
        for u in range(UNITS):
            r0 = (8 * h + u) * PAIR_ROWS
            out[b, r0 : r0 + PAIR_ROWS, :] = res.results[c]["out"][u]
    return out
